# revision 5
# baseline (speedup 1.0000x reference)
"""Causal self-attention Trainium2 kernel, v2 (transpose-free attention).

B=1024, S=77, E=1024, H=16, D=64. Data-parallel over batch across 8 cores
(128 batches/core). bf16 on the PE with fp32 PSUM accumulation.

v2 dataflow change vs v1: compute scores TRANSPOSED from the start
(S^T[k,q] = K @ Q^T via lhsT=K^T chunk, rhs=Q^T chunk), so the attention
matrix never needs a PE transpose:

  S^T  [77k, 77q]   PSUM   (even/odd heads in separate banks, row groups)
  t3   = S^T + mask^T      (DVE, causal mask, -1e30 above diag^T)
  eb   = exp(t3)           (ACT -> SBUF bf16)
  z    [1, 462]     PSUM   = ones[77]^T @ eb        (PE, one matmul/group)
  zr   = 1/z               (DVE -> SBUF bf16)
  zrep [77, 462]    PSUM   = ones[77] outer zr      (PE, K=1 matmul)
  attT = eb * zrep         (DVE -> SBUF bf16)
  O^T  [128f, 77q]  PSUM   = V_h^T-free matmul: lhsT=vb[g][:,64h:64h+64],
                            rhs=attT slice (col-pair packed, tile_position)
  proj unchanged.

This removes 96 PE transposes + 16 ACT copies per 6-batch block at the cost
of 32 cheap N=462 matmuls (z + zrep).
"""

import sys

sys.path.insert(0, "/opt/trn_rl_repo")

import numpy as np
import ml_dtypes

import concourse.bass as bass
import concourse.mybir as mybir
import concourse.tile as tile
from concourse import bacc
from concourse.bass_utils import run_bass_kernel_spmd

F32 = mybir.dt.float32
BF16 = mybir.dt.bfloat16
AX = mybir.AxisListType
AF = mybir.ActivationFunctionType

N_CORES = 8
B, S, E = 1024, 77, 1024
H, D = 16, 64
BC = B // N_CORES          # batches per core = 128
T = BC * S                 # tokens per core = 9856
SCALE = 1.0 / float(np.sqrt(D))
NEG = -1.0e30

# block structure: 21 blocks of 6 batches + 1 block of 2
BLOCKS = [(i * 6, 6) for i in range(21)] + [(126, 2)]


def _load_x(nc, P, b0, G):
    Tb = G * S
    t0 = b0 * S
    xt = []
    for e in range(8):
        xtile = P["x"].tile([128, Tb], BF16, tag=f"xt{e}")
        nc.sync.dma_start(xtile[:], P["xT"][128 * e:128 * (e + 1), t0:t0 + Tb])
        xt.append(xtile)
    return xt


def _emit_block(nc, tc, P, b0, G, xt=None):
    Tb = G * S                       # tokens this block
    t0 = b0 * S
    if xt is None:
        xt = _load_x(nc, P, b0, G)

    # ---- Q^T / K^T GEMM: 16 f-chunks of 128, contraction over 8 e-chunks
    qk = []
    for c in range(16):
        ps = P["gps"].tile([128, 512], F32, tag="g")
        for e in range(8):
            nc.tensor.matmul(
                ps[:, :Tb],
                P["wqk"][e][:, 128 * c:128 * (c + 1)],
                xt[e][:],
                start=(e == 0), stop=(e == 7),
            )
        o = P["qk"].tile([128, Tb], BF16, tag=f"qk{c}")
        # Identity(ps*scale + bias): SCALE folded into Q here (bias pre-scaled on host)
        nc.scalar.activation(
            o[:], ps[:, :Tb], AF.Identity,
            bias=P["bqk"][:, c:c + 1], scale=(SCALE if c < 8 else 1.0),
        )
        qk.append(o)

    # ---- V GEMM per batch: out [77 tokens, 1024 f]
    vb = []
    for g in range(G):
        v = P["v"].tile([77, 1024], BF16, tag=f"v{g}")
        for fc in range(2):
            ps = P["gps"].tile([128, 512], F32, tag="g")
            for e in range(8):
                nc.tensor.matmul(
                    ps[:77, :],
                    xt[e][:, S * g:S * (g + 1)],
                    P["wv"][e][:, 512 * fc:512 * (fc + 1)],
                    start=(e == 0), stop=(e == 7),
                )
            nc.scalar.activation(v[:, 512 * fc:512 * (fc + 1)], ps[:77, :], AF.Copy)
        vb.append(v)

    # ---- attention, transposed scores. HW constraint: matmuls within one
    # PSUM bank must not alternate row groups -> even heads (array rows 0:64)
    # and odd heads (rows 64:128) accumulate in separate score banks, emitted
    # interleaved so PE overlaps LDWEIGHTS across row groups.
    def _softmax_T(sc, grp):
        n = len(grp)
        W = n * S
        t3 = P["sm"].tile([77, 462], F32, tag="tsb", name="tsb")
        nc.vector.tensor_add(t3[:, :W], sc[:, :W], P["maskT"][:77, :W])

        eb = P["sm"].tile([77, 462], BF16, tag="esb", name="esb")
        nc.scalar.activation(eb[:, :W], t3[:, :W], AF.Exp)
        # z[1, W] = column sums of eb (reduce along partitions via PE);
        # shares the score pool's PSUM banks (short-lived)
        zps = P["scps"].tile([1, 462], F32, tag="sc", name="zps")
        nc.tensor.matmul(
            zps[:, :W], P["ones"][:77, 0:1], eb[:, :W], start=True, stop=True,
        )
        zr = P["zr"].tile([1, 462], BF16, tag="zr", name="zr")
        with nc.allow_low_precision(reason="1/z in bf16: feeds bf16 att weights"):
            nc.vector.reciprocal(zr[:, :W], zps[:, :W])
        # replicate 1/z down the k-partitions on the otherwise-idle GpSimd
        zrep = P["zrep"].tile([77, 462], BF16, tag="zrep", name="zrep")
        nc.gpsimd.partition_broadcast(zrep[:77, :W], zr[0:1, :W])
        aT = P["attT"].tile([77, 462], BF16, tag="attT", name="attT")
        nc.gpsimd.tensor_mul(aT[:, :W], eb[:, :W], zrep[:77, :W])
        return aT

    evens = [(g, h) for g in range(G) for h in range(0, H, 2)]
    odds = [(g, h) for g in range(G) for h in range(1, H, 2)]
    egroups = [evens[i:i + 6] for i in range(0, len(evens), 6)]
    ogroups = [odds[i:i + 6] for i in range(0, len(odds), 6)]
    attT = []
    pair_loc = {}
    for eg, og in zip(egroups, ogroups):
        scA_f = P["scps"].tile([128, 512], F32, tag="sc", name="sc")
        scB_f = P["scps"].tile([128, 512], F32, tag="sc", name="sc")
        scA, scB = scA_f[:77, :], scB_f[:77, :]
        for i in range(len(eg)):
            gA, hA = eg[i]
            gB, hB = og[i]
            cA, cB = hA // 2, hB // 2
            # S^T[k, q] = K @ Q^T : lhsT = K^T chunk, rhs = Q^T chunk
            nc.tensor.matmul(
                scA[:, S * i:S * (i + 1)],
                qk[8 + cA][0:64, S * gA:S * (gA + 1)],
                qk[cA][0:64, S * gA:S * (gA + 1)],
                start=True, stop=True,
            )
            nc.tensor.matmul(
                scB[:, S * i:S * (i + 1)],
                qk[8 + cB][64:128, S * gB:S * (gB + 1)],
                qk[cB][64:128, S * gB:S * (gB + 1)],
                start=True, stop=True,
            )
        aT_A = _softmax_T(scA, eg)
        for i, pr in enumerate(eg):
            pair_loc[pr] = (len(attT), i)
        attT.append(aT_A)
        aT_B = _softmax_T(scB, og)
        for i, pr in enumerate(og):
            pair_loc[pr] = (len(attT), i)
        attT.append(aT_B)

    # ---- O^T: head-pair col-packed matmuls, PSUM bank per pair-index j
    ot = []
    for j in range(8):
        ps2 = P["m2ps"].tile([128, 512], F32, tag="m2", name="m2")
        for g in range(G):
            giE, slE = pair_loc[(g, 2 * j)]
            giO, slO = pair_loc[(g, 2 * j + 1)]
            nc.tensor.matmul(
                ps2[0:64, S * g:S * (g + 1)],
                vb[g][:, 64 * (2 * j):64 * (2 * j) + 64],
                attT[giE][:, S * slE:S * (slE + 1)],
                start=True, stop=True,
            )
            nc.tensor.matmul(
                ps2[64:128, S * g:S * (g + 1)],
                vb[g][:, 64 * (2 * j + 1):64 * (2 * j + 1) + 64],
                attT[giO][:, S * slO:S * (slO + 1)],
                start=True, stop=True,
                tile_position=(0, 64),
            )
        o = P["ot"].tile([128, Tb], BF16, tag=f"ot{j}")
        nc.scalar.activation(
            o[:], ps2[:, :Tb], AF.Identity, bias=P["bv"][:, j:j + 1]
        )
        ot.append(o)

    # ---- projection: y^T[e-chunk, t] = sum_j Wp[j-chunk]^T @ O^T[j]
    # (uses the m2 pool, not gps: keeps next block's QK GEMM from queuing
    # behind proj in the gps slot-request FIFO)
    for ec in range(8):
        ps = P["m2ps"].tile([128, 512], F32, tag="m2")
        for j in range(8):
            nc.tensor.matmul(
                ps[:, :Tb],
                P["wp"][j][:, 128 * ec:128 * (ec + 1)],
                ot[j][:],
                start=(j == 0), stop=(j == 7),
            )
        y = P["y"].tile([128, Tb], F32, tag="y")
        nc.scalar.activation(
            y[:], ps[:, :Tb], AF.Identity, bias=P["bp"][:, ec:ec + 1]
        )
        nc.sync.dma_start(P["yT"][128 * ec:128 * (ec + 1), t0:t0 + Tb], y[:])


def build(blocks=None, repeat=1):
    if blocks is None:
        blocks = BLOCKS
    nc = bacc.Bacc(None)
    xT = nc.dram_tensor("xT", [E, T], BF16, kind="ExternalInput")
    wqk_d = nc.dram_tensor("wqk", [E, 2048], BF16, kind="ExternalInput")
    wv_d = nc.dram_tensor("wv", [E, 1024], BF16, kind="ExternalInput")
    wp_d = nc.dram_tensor("wp", [1024, 1024], BF16, kind="ExternalInput")
    bqk_d = nc.dram_tensor("bqk", [128, 16], F32, kind="ExternalInput")
    bv_d = nc.dram_tensor("bv", [128, 8], F32, kind="ExternalInput")
    bp_d = nc.dram_tensor("bp", [128, 8], F32, kind="ExternalInput")
    maskT_d = nc.dram_tensor("maskT", [77, 462], F32, kind="ExternalInput")
    ones_d = nc.dram_tensor("ones", [77, 77], BF16, kind="ExternalInput")
    yT = nc.dram_tensor("yT", [E, T], F32, kind="ExternalOutput")

    with tile.TileContext(nc) as tc:
        with (
            tc.tile_pool(name="w", bufs=1) as wpool,
            tc.tile_pool(name="x", bufs=2) as xpool,
            tc.tile_pool(name="qk", bufs=2) as qkpool,
            tc.tile_pool(name="v", bufs=2) as vpool,
            tc.tile_pool(name="sm", bufs=3) as smpool,
            tc.tile_pool(name="zr", bufs=3) as zrpool,
            tc.tile_pool(name="zrep", bufs=3) as zreppool,
            tc.tile_pool(name="attT", bufs=18) as attTpool,
            tc.tile_pool(name="ot", bufs=2) as otpool,
            tc.tile_pool(name="y", bufs=3) as ypool,
            tc.tile_pool(name="gps", bufs=2, space="PSUM") as gpspool,
            tc.tile_pool(name="scps", bufs=4, space="PSUM") as scpool,
            tc.tile_pool(name="m2ps", bufs=2, space="PSUM") as m2pool,
        ):
            P = {}
            # DMA order matters: small constants + first QK weight chunk
            # first so block 0's x tiles (emitted next, in _emit_block) don't
            # queue behind 8MB of weights; remaining weights stream in while
            # block 0's QK GEMM runs.
            P["bqk"] = wpool.tile([128, 16], F32, tag="bqk", name="bqk")
            nc.sync.dma_start(P["bqk"][:], bqk_d[:])
            P["bv"] = wpool.tile([128, 8], F32, tag="bv", name="bv")
            nc.sync.dma_start(P["bv"][:], bv_d[:])
            P["bp"] = wpool.tile([128, 8], F32, tag="bp", name="bp")
            nc.sync.dma_start(P["bp"][:], bp_d[:])
            P["maskT"] = wpool.tile([77, 462], F32, tag="maskT", name="maskT")
            nc.sync.dma_start(P["maskT"][:], maskT_d[:])
            P["ones"] = wpool.tile([77, 77], BF16, tag="ones", name="ones")
            nc.sync.dma_start(P["ones"][:], ones_d[:])
            P["xT"] = xT
            P["x"] = xpool
            xt0 = _load_x(nc, P, blocks[0][0], blocks[0][1]) if repeat == 1 else None
            P["wqk"] = []
            P["wv"] = []
            P["wp"] = []
            for e in range(8):
                w1 = wpool.tile([128, 2048], BF16, tag=f"wqk{e}", name=f"wqk{e}")
                nc.sync.dma_start(w1[:], wqk_d[128 * e:128 * (e + 1), :])
                P["wqk"].append(w1)
            for e in range(8):
                w2 = wpool.tile([128, 1024], BF16, tag=f"wv{e}", name=f"wv{e}")
                nc.sync.dma_start(w2[:], wv_d[128 * e:128 * (e + 1), :])
                P["wv"].append(w2)
            for e in range(8):
                w3 = wpool.tile([128, 1024], BF16, tag=f"wp{e}", name=f"wp{e}")
                nc.sync.dma_start(w3[:], wp_d[128 * e:128 * (e + 1), :])
                P["wp"].append(w3)
            P["yT"] = yT
            P["qk"] = qkpool
            P["v"] = vpool
            P["sm"] = smpool
            P["zr"] = zrpool
            P["zrep"] = zreppool
            P["attT"] = attTpool
            P["ot"] = otpool
            P["y"] = ypool
            P["gps"] = gpspool
            P["scps"] = scpool
            P["m2ps"] = m2pool

            def body(first_xt=None):
                for bi, (b0, G) in enumerate(blocks):
                    _emit_block(nc, tc, P, b0, G,
                                xt=first_xt if bi == 0 else None)

            if repeat == 1:
                body(first_xt=xt0)
            else:
                # first iteration reuses the prefetched x tiles; the loop
                # reloads them each pass (identical work every iteration)
                with tc.For_i(0, repeat):
                    body()

    nc.finalize()
    return nc


_CACHE = {}


def _get_nc():
    if "nc" not in _CACHE:
        _CACHE["nc"] = build()
    return _CACHE["nc"]


def make_inputs(x, W_attn, b_attn, W_proj, b_proj):
    """Host-side prep: shard + transpose + cast. Returns per-core input maps."""
    x = np.asarray(x, dtype=np.float32)
    W_attn = np.asarray(W_attn, dtype=np.float32)
    b_attn = np.asarray(b_attn, dtype=np.float32)
    W_proj = np.asarray(W_proj, dtype=np.float32)
    b_proj = np.asarray(b_proj, dtype=np.float32)

    wqk = W_attn[:, :2048].astype(ml_dtypes.bfloat16)
    wv = W_attn[:, 2048:].astype(ml_dtypes.bfloat16)
    wp = W_proj.astype(ml_dtypes.bfloat16)
    # bias chunks [128, 16]: col c = b_attn[128c:128c+128]; Q part pre-scaled
    bq = b_attn[:2048].copy()
    bq[:1024] *= SCALE
    bqk = np.stack([bq[128 * c:128 * (c + 1)] for c in range(16)], axis=1).astype(np.float32)
    bv = np.stack([b_attn[2048 + 128 * j:2048 + 128 * (j + 1)] for j in range(8)], axis=1).astype(np.float32)
    bp = np.stack([b_proj[128 * c:128 * (c + 1)] for c in range(8)], axis=1).astype(np.float32)
    # transposed causal mask: maskT[k, q] = 0 if k <= q else NEG
    maskT = np.where(
        np.triu(np.ones((77, 77), dtype=bool)), 0.0, NEG
    ).astype(np.float32)
    maskT = np.tile(maskT, (1, 6))  # dense [77, 462]: one slot per group pair
    ones = np.ones((77, 77), dtype=ml_dtypes.bfloat16)

    maps = []
    for cid in range(N_CORES):
        xs = x[BC * cid:BC * (cid + 1)].reshape(T, E)
        xTc = np.ascontiguousarray(xs.T).astype(ml_dtypes.bfloat16)
        maps.append({
            "xT": xTc, "wqk": wqk, "wv": wv, "wp": wp,
            "bqk": bqk, "bv": bv, "bp": bp, "maskT": maskT, "ones": ones,
        })
    return maps


def assemble_output(results):
    y = np.empty((B, S, E), dtype=np.float32)
    for cid in range(N_CORES):
        yTc = results[cid]["yT"]  # [E, T]
        y[BC * cid:BC * (cid + 1)] = yTc.T.reshape(BC, S, E)
    return y


def kernel(x, W_attn, b_attn, W_proj, b_proj):
    nc = _get_nc()
    maps = make_inputs(x, W_attn, b_attn, W_proj, b_proj)
    res = run_bass_kernel_spmd(nc, maps, list(range(N_CORES)))
    return assemble_output(res.results)


# revision 6
# speedup vs baseline: 2.1769x; 2.1769x over previous
"""Causal self-attention Trainium2 kernel, v2 (transpose-free attention).

B=1024, S=77, E=1024, H=16, D=64. Data-parallel over batch across 8 cores
(128 batches/core). bf16 on the PE with fp32 PSUM accumulation.

v2 dataflow change vs v1: compute scores TRANSPOSED from the start
(S^T[k,q] = K @ Q^T via lhsT=K^T chunk, rhs=Q^T chunk), so the attention
matrix never needs a PE transpose:

  S^T  [77k, 77q]   PSUM   (even/odd heads in separate banks, row groups)
  t3   = S^T + mask^T      (DVE, causal mask, -1e30 above diag^T)
  eb   = exp(t3)           (ACT -> SBUF bf16)
  z    [1, 462]     PSUM   = ones[77]^T @ eb        (PE, one matmul/group)
  zr   = 1/z               (DVE -> SBUF bf16)
  zrep [77, 462]    PSUM   = ones[77] outer zr      (PE, K=1 matmul)
  attT = eb * zrep         (DVE -> SBUF bf16)
  O^T  [128f, 77q]  PSUM   = V_h^T-free matmul: lhsT=vb[g][:,64h:64h+64],
                            rhs=attT slice (col-pair packed, tile_position)
  proj unchanged.

This removes 96 PE transposes + 16 ACT copies per 6-batch block at the cost
of 32 cheap N=462 matmuls (z + zrep).
"""

import sys

sys.path.insert(0, "/opt/trn_rl_repo")

import numpy as np
import ml_dtypes

import concourse.bass as bass
import concourse.mybir as mybir
import concourse.tile as tile
from concourse import bacc
from concourse.bass_utils import run_bass_kernel_spmd

F32 = mybir.dt.float32
BF16 = mybir.dt.bfloat16
AX = mybir.AxisListType
AF = mybir.ActivationFunctionType

N_CORES = 8
B, S, E = 1024, 77, 1024
H, D = 16, 64
BC = B // N_CORES          # batches per core = 128
T = BC * S                 # tokens per core = 9856
SCALE = 1.0 / float(np.sqrt(D))
NEG = -1.0e30

# block structure: 21 blocks of 6 batches + 1 block of 2
BLOCKS = [(i * 6, 6) for i in range(21)] + [(126, 2)]


def _load_x(nc, P, b0, G):
    Tb = G * S
    t0 = b0 * S
    xt = []
    for e in range(8):
        xtile = P["x"].tile([128, Tb], BF16, tag=f"xt{e}")
        nc.sync.dma_start(xtile[:], P["xT"][128 * e:128 * (e + 1), t0:t0 + Tb])
        xt.append(xtile)
    return xt


def _emit_block(nc, tc, P, b0, G, xt=None):
    Tb = G * S                       # tokens this block
    t0 = b0 * S
    if xt is None:
        xt = _load_x(nc, P, b0, G)

    # ---- Q^T / K^T GEMM: 16 f-chunks of 128, contraction over 8 e-chunks
    qk = []
    for c in range(16):
        ps = P["gps"].tile([128, 512], F32, tag="g")
        for e in range(8):
            nc.tensor.matmul(
                ps[:, :Tb],
                P["wqk"][e][:, 128 * c:128 * (c + 1)],
                xt[e][:],
                start=(e == 0), stop=(e == 7),
            )
        o = P["qk"].tile([128, Tb], BF16, tag=f"qk{c}")
        # Identity(ps*scale + bias): SCALE folded into Q here (bias pre-scaled on host)
        nc.scalar.activation(
            o[:], ps[:, :Tb], AF.Identity,
            bias=P["bqk"][:, c:c + 1], scale=(SCALE if c < 8 else 1.0),
        )
        qk.append(o)

    # ---- V GEMM per batch: out [77 tokens, 1024 f]
    vb = []
    for g in range(G):
        v = P["v"].tile([77, 1024], BF16, tag=f"v{g}")
        for fc in range(2):
            ps = P["gps"].tile([128, 512], F32, tag="g")
            for e in range(8):
                nc.tensor.matmul(
                    ps[:77, :],
                    xt[e][:, S * g:S * (g + 1)],
                    P["wv"][e][:, 512 * fc:512 * (fc + 1)],
                    start=(e == 0), stop=(e == 7),
                )
            nc.scalar.activation(v[:, 512 * fc:512 * (fc + 1)], ps[:77, :], AF.Copy)
        vb.append(v)

    # ---- attention, transposed scores. HW constraint: matmuls within one
    # PSUM bank must not alternate row groups -> even heads (array rows 0:64)
    # and odd heads (rows 64:128) accumulate in separate score banks, emitted
    # interleaved so PE overlaps LDWEIGHTS across row groups.
    def _softmax_T(sc, grp):
        n = len(grp)
        W = n * S
        t3 = P["sm"].tile([77, 462], F32, tag="tsb", name="tsb")
        nc.vector.tensor_add(t3[:, :W], sc[:, :W], P["maskT"][:77, :W])

        eb = P["sm"].tile([77, 462], BF16, tag="esb", name="esb")
        nc.scalar.activation(eb[:, :W], t3[:, :W], AF.Exp)
        # z[1, W] = column sums of eb (reduce along partitions via PE);
        # shares the score pool's PSUM banks (short-lived)
        zps = P["scps"].tile([1, 462], F32, tag="sc", name="zps")
        nc.tensor.matmul(
            zps[:, :W], P["ones"][:77, 0:1], eb[:, :W], start=True, stop=True,
        )
        zr = P["zr"].tile([1, 462], BF16, tag="zr", name="zr")
        with nc.allow_low_precision(reason="1/z in bf16: feeds bf16 att weights"):
            nc.vector.reciprocal(zr[:, :W], zps[:, :W])
        # replicate 1/z down the k-partitions on the otherwise-idle GpSimd
        zrep = P["zrep"].tile([77, 462], BF16, tag="zrep", name="zrep")
        nc.gpsimd.partition_broadcast(zrep[:77, :W], zr[0:1, :W])
        aT = P["attT"].tile([77, 462], BF16, tag="attT", name="attT")
        nc.vector.tensor_mul(aT[:, :W], eb[:, :W], zrep[:77, :W])
        return aT

    evens = [(g, h) for g in range(G) for h in range(0, H, 2)]
    odds = [(g, h) for g in range(G) for h in range(1, H, 2)]
    egroups = [evens[i:i + 6] for i in range(0, len(evens), 6)]
    ogroups = [odds[i:i + 6] for i in range(0, len(odds), 6)]
    attT = []
    pair_loc = {}
    for eg, og in zip(egroups, ogroups):
        scA_f = P["scps"].tile([128, 512], F32, tag="sc", name="sc")
        scB_f = P["scps"].tile([128, 512], F32, tag="sc", name="sc")
        scA, scB = scA_f[:77, :], scB_f[:77, :]
        for i in range(len(eg)):
            gA, hA = eg[i]
            gB, hB = og[i]
            cA, cB = hA // 2, hB // 2
            # S^T[k, q] = K @ Q^T : lhsT = K^T chunk, rhs = Q^T chunk
            nc.tensor.matmul(
                scA[:, S * i:S * (i + 1)],
                qk[8 + cA][0:64, S * gA:S * (gA + 1)],
                qk[cA][0:64, S * gA:S * (gA + 1)],
                start=True, stop=True,
            )
            nc.tensor.matmul(
                scB[:, S * i:S * (i + 1)],
                qk[8 + cB][64:128, S * gB:S * (gB + 1)],
                qk[cB][64:128, S * gB:S * (gB + 1)],
                start=True, stop=True,
            )
        aT_A = _softmax_T(scA, eg)
        for i, pr in enumerate(eg):
            pair_loc[pr] = (len(attT), i)
        attT.append(aT_A)
        aT_B = _softmax_T(scB, og)
        for i, pr in enumerate(og):
            pair_loc[pr] = (len(attT), i)
        attT.append(aT_B)

    # ---- O^T: head-pair col-packed matmuls, PSUM bank per pair-index j
    ot = []
    for j in range(8):
        ps2 = P["m2ps"].tile([128, 512], F32, tag="m2", name="m2")
        for g in range(G):
            giE, slE = pair_loc[(g, 2 * j)]
            giO, slO = pair_loc[(g, 2 * j + 1)]
            nc.tensor.matmul(
                ps2[0:64, S * g:S * (g + 1)],
                vb[g][:, 64 * (2 * j):64 * (2 * j) + 64],
                attT[giE][:, S * slE:S * (slE + 1)],
                start=True, stop=True,
            )
            nc.tensor.matmul(
                ps2[64:128, S * g:S * (g + 1)],
                vb[g][:, 64 * (2 * j + 1):64 * (2 * j + 1) + 64],
                attT[giO][:, S * slO:S * (slO + 1)],
                start=True, stop=True,
                tile_position=(0, 64),
            )
        o = P["ot"].tile([128, Tb], BF16, tag=f"ot{j}")
        nc.scalar.activation(
            o[:], ps2[:, :Tb], AF.Identity, bias=P["bv"][:, j:j + 1]
        )
        ot.append(o)

    # ---- projection: y^T[e-chunk, t] = sum_j Wp[j-chunk]^T @ O^T[j]
    # (uses the m2 pool, not gps: keeps next block's QK GEMM from queuing
    # behind proj in the gps slot-request FIFO)
    for ec in range(8):
        ps = P["m2ps"].tile([128, 512], F32, tag="m2")
        for j in range(8):
            nc.tensor.matmul(
                ps[:, :Tb],
                P["wp"][j][:, 128 * ec:128 * (ec + 1)],
                ot[j][:],
                start=(j == 0), stop=(j == 7),
            )
        y = P["y"].tile([128, Tb], F32, tag="y")
        nc.scalar.activation(
            y[:], ps[:, :Tb], AF.Identity, bias=P["bp"][:, ec:ec + 1]
        )
        nc.sync.dma_start(P["yT"][128 * ec:128 * (ec + 1), t0:t0 + Tb], y[:])


def build(blocks=None, repeat=1):
    if blocks is None:
        blocks = BLOCKS
    nc = bacc.Bacc(None)
    xT = nc.dram_tensor("xT", [E, T], BF16, kind="ExternalInput")
    wqk_d = nc.dram_tensor("wqk", [E, 2048], BF16, kind="ExternalInput")
    wv_d = nc.dram_tensor("wv", [E, 1024], BF16, kind="ExternalInput")
    wp_d = nc.dram_tensor("wp", [1024, 1024], BF16, kind="ExternalInput")
    bqk_d = nc.dram_tensor("bqk", [128, 16], F32, kind="ExternalInput")
    bv_d = nc.dram_tensor("bv", [128, 8], F32, kind="ExternalInput")
    bp_d = nc.dram_tensor("bp", [128, 8], F32, kind="ExternalInput")
    maskT_d = nc.dram_tensor("maskT", [77, 462], F32, kind="ExternalInput")
    ones_d = nc.dram_tensor("ones", [77, 77], BF16, kind="ExternalInput")
    yT = nc.dram_tensor("yT", [E, T], F32, kind="ExternalOutput")

    with tile.TileContext(nc) as tc:
        with (
            tc.tile_pool(name="w", bufs=1) as wpool,
            tc.tile_pool(name="x", bufs=2) as xpool,
            tc.tile_pool(name="qk", bufs=2) as qkpool,
            tc.tile_pool(name="v", bufs=2) as vpool,
            tc.tile_pool(name="sm", bufs=3) as smpool,
            tc.tile_pool(name="zr", bufs=3) as zrpool,
            tc.tile_pool(name="zrep", bufs=3) as zreppool,
            tc.tile_pool(name="attT", bufs=18) as attTpool,
            tc.tile_pool(name="ot", bufs=2) as otpool,
            tc.tile_pool(name="y", bufs=3) as ypool,
            tc.tile_pool(name="gps", bufs=2, space="PSUM") as gpspool,
            tc.tile_pool(name="scps", bufs=4, space="PSUM") as scpool,
            tc.tile_pool(name="m2ps", bufs=2, space="PSUM") as m2pool,
        ):
            P = {}
            # DMA order matters: small constants + first QK weight chunk
            # first so block 0's x tiles (emitted next, in _emit_block) don't
            # queue behind 8MB of weights; remaining weights stream in while
            # block 0's QK GEMM runs.
            P["bqk"] = wpool.tile([128, 16], F32, tag="bqk", name="bqk")
            nc.sync.dma_start(P["bqk"][:], bqk_d[:])
            P["bv"] = wpool.tile([128, 8], F32, tag="bv", name="bv")
            nc.sync.dma_start(P["bv"][:], bv_d[:])
            P["bp"] = wpool.tile([128, 8], F32, tag="bp", name="bp")
            nc.sync.dma_start(P["bp"][:], bp_d[:])
            P["maskT"] = wpool.tile([77, 462], F32, tag="maskT", name="maskT")
            nc.sync.dma_start(P["maskT"][:], maskT_d[:])
            P["ones"] = wpool.tile([77, 77], BF16, tag="ones", name="ones")
            nc.sync.dma_start(P["ones"][:], ones_d[:])
            P["xT"] = xT
            P["x"] = xpool
            xt0 = _load_x(nc, P, blocks[0][0], blocks[0][1]) if repeat == 1 else None
            P["wqk"] = []
            P["wv"] = []
            P["wp"] = []
            for e in range(8):
                w1 = wpool.tile([128, 2048], BF16, tag=f"wqk{e}", name=f"wqk{e}")
                nc.sync.dma_start(w1[:], wqk_d[128 * e:128 * (e + 1), :])
                P["wqk"].append(w1)
            for e in range(8):
                w2 = wpool.tile([128, 1024], BF16, tag=f"wv{e}", name=f"wv{e}")
                nc.sync.dma_start(w2[:], wv_d[128 * e:128 * (e + 1), :])
                P["wv"].append(w2)
            for e in range(8):
                w3 = wpool.tile([128, 1024], BF16, tag=f"wp{e}", name=f"wp{e}")
                nc.sync.dma_start(w3[:], wp_d[128 * e:128 * (e + 1), :])
                P["wp"].append(w3)
            P["yT"] = yT
            P["qk"] = qkpool
            P["v"] = vpool
            P["sm"] = smpool
            P["zr"] = zrpool
            P["zrep"] = zreppool
            P["attT"] = attTpool
            P["ot"] = otpool
            P["y"] = ypool
            P["gps"] = gpspool
            P["scps"] = scpool
            P["m2ps"] = m2pool

            def body(first_xt=None):
                for bi, (b0, G) in enumerate(blocks):
                    _emit_block(nc, tc, P, b0, G,
                                xt=first_xt if bi == 0 else None)

            if repeat == 1:
                body(first_xt=xt0)
            else:
                # first iteration reuses the prefetched x tiles; the loop
                # reloads them each pass (identical work every iteration)
                with tc.For_i(0, repeat):
                    body()

    nc.finalize()
    return nc


_CACHE = {}


def _get_nc():
    if "nc" not in _CACHE:
        _CACHE["nc"] = build()
    return _CACHE["nc"]


def make_inputs(x, W_attn, b_attn, W_proj, b_proj):
    """Host-side prep: shard + transpose + cast. Returns per-core input maps."""
    x = np.asarray(x, dtype=np.float32)
    W_attn = np.asarray(W_attn, dtype=np.float32)
    b_attn = np.asarray(b_attn, dtype=np.float32)
    W_proj = np.asarray(W_proj, dtype=np.float32)
    b_proj = np.asarray(b_proj, dtype=np.float32)

    wqk = W_attn[:, :2048].astype(ml_dtypes.bfloat16)
    wv = W_attn[:, 2048:].astype(ml_dtypes.bfloat16)
    wp = W_proj.astype(ml_dtypes.bfloat16)
    # bias chunks [128, 16]: col c = b_attn[128c:128c+128]; Q part pre-scaled
    bq = b_attn[:2048].copy()
    bq[:1024] *= SCALE
    bqk = np.stack([bq[128 * c:128 * (c + 1)] for c in range(16)], axis=1).astype(np.float32)
    bv = np.stack([b_attn[2048 + 128 * j:2048 + 128 * (j + 1)] for j in range(8)], axis=1).astype(np.float32)
    bp = np.stack([b_proj[128 * c:128 * (c + 1)] for c in range(8)], axis=1).astype(np.float32)
    # transposed causal mask: maskT[k, q] = 0 if k <= q else NEG
    maskT = np.where(
        np.triu(np.ones((77, 77), dtype=bool)), 0.0, NEG
    ).astype(np.float32)
    maskT = np.tile(maskT, (1, 6))  # dense [77, 462]: one slot per group pair
    ones = np.ones((77, 77), dtype=ml_dtypes.bfloat16)

    maps = []
    for cid in range(N_CORES):
        xs = x[BC * cid:BC * (cid + 1)].reshape(T, E)
        xTc = np.ascontiguousarray(xs.T).astype(ml_dtypes.bfloat16)
        maps.append({
            "xT": xTc, "wqk": wqk, "wv": wv, "wp": wp,
            "bqk": bqk, "bv": bv, "bp": bp, "maskT": maskT, "ones": ones,
        })
    return maps


def assemble_output(results):
    y = np.empty((B, S, E), dtype=np.float32)
    for cid in range(N_CORES):
        yTc = results[cid]["yT"]  # [E, T]
        y[BC * cid:BC * (cid + 1)] = yTc.T.reshape(BC, S, E)
    return y


def kernel(x, W_attn, b_attn, W_proj, b_proj):
    nc = _get_nc()
    maps = make_inputs(x, W_attn, b_attn, W_proj, b_proj)
    res = run_bass_kernel_spmd(nc, maps, list(range(N_CORES)))
    return assemble_output(res.results)


# revision 8
# speedup vs baseline: 2.4345x; 1.1183x over previous
"""Causal self-attention Trainium2 kernel, v2 (transpose-free attention).

B=1024, S=77, E=1024, H=16, D=64. Data-parallel over batch across 8 cores
(128 batches/core). bf16 on the PE with fp32 PSUM accumulation.

v2 dataflow change vs v1: compute scores TRANSPOSED from the start
(S^T[k,q] = K @ Q^T via lhsT=K^T chunk, rhs=Q^T chunk), so the attention
matrix never needs a PE transpose:

  S^T  [77k, 77q]   PSUM   (even/odd heads in separate banks, row groups)
  t3   = S^T + mask^T      (DVE, causal mask, -1e30 above diag^T)
  eb   = exp(t3)           (ACT -> SBUF bf16)
  z    [1, 462]     PSUM   = ones[77]^T @ eb        (PE, one matmul/group)
  zr   = 1/z               (DVE -> SBUF bf16)
  zrep [77, 462]    PSUM   = ones[77] outer zr      (PE, K=1 matmul)
  attT = eb * zrep         (DVE -> SBUF bf16)
  O^T  [128f, 77q]  PSUM   = V_h^T-free matmul: lhsT=vb[g][:,64h:64h+64],
                            rhs=attT slice (col-pair packed, tile_position)
  proj unchanged.

This removes 96 PE transposes + 16 ACT copies per 6-batch block at the cost
of 32 cheap N=462 matmuls (z + zrep).
"""

import sys

sys.path.insert(0, "/opt/trn_rl_repo")

import numpy as np
import ml_dtypes

import concourse.bass as bass
import concourse.mybir as mybir
import concourse.tile as tile
from concourse import bacc
from concourse.bass_utils import run_bass_kernel_spmd

F32 = mybir.dt.float32
BF16 = mybir.dt.bfloat16
AX = mybir.AxisListType
AF = mybir.ActivationFunctionType

N_CORES = 8
B, S, E = 1024, 77, 1024
H, D = 16, 64
BC = B // N_CORES          # batches per core = 128
T = BC * S                 # tokens per core = 9856
SCALE = 1.0 / float(np.sqrt(D))
NEG = -1.0e30

# block structure: 21 blocks of 6 batches + 1 block of 2
BLOCKS = [(i * 6, 6) for i in range(21)] + [(126, 2)]


def _load_x(nc, P, b0, G):
    Tb = G * S
    t0 = b0 * S
    xt = []
    for e in range(8):
        xtile = P["x"].tile([128, Tb], BF16, tag=f"xt{e}")
        nc.sync.dma_start(xtile[:], P["xT"][128 * e:128 * (e + 1), t0:t0 + Tb])
        xt.append(xtile)
    return xt


def _emit_block(nc, tc, P, b0, G, xt=None):
    Tb = G * S                       # tokens this block
    t0 = b0 * S
    if xt is None:
        xt = _load_x(nc, P, b0, G)

    # ---- Q^T / K^T GEMM: 16 f-chunks of 128, contraction over 8 e-chunks
    qk = []
    for c in range(16):
        ps = P["gps"].tile([128, 512], F32, tag="g")
        for e in range(8):
            nc.tensor.matmul(
                ps[:, :Tb],
                P["wqk"][e][:, 128 * c:128 * (c + 1)],
                xt[e][:],
                start=(e == 0), stop=(e == 7),
            )
        o = P["qk"].tile([128, Tb], BF16, tag=f"qk{c}")
        # Identity(ps*scale + bias): SCALE folded into Q here (bias pre-scaled on host)
        nc.scalar.activation(
            o[:], ps[:, :Tb], AF.Identity,
            bias=P["bqk"][:, c:c + 1], scale=(SCALE if c < 8 else 1.0),
        )
        qk.append(o)

    # ---- V GEMM per batch: out [77 tokens, 1024 f]
    vb = []
    for g in range(G):
        v = P["v"].tile([77, 1024], BF16, tag=f"v{g}")
        for fc in range(2):
            ps = P["gps"].tile([128, 512], F32, tag="g")
            for e in range(8):
                nc.tensor.matmul(
                    ps[:77, :],
                    xt[e][:, S * g:S * (g + 1)],
                    P["wv"][e][:, 512 * fc:512 * (fc + 1)],
                    start=(e == 0), stop=(e == 7),
                )
            nc.vector.tensor_copy(v[:, 512 * fc:512 * (fc + 1)], ps[:77, :])
        vb.append(v)

    # ---- attention, transposed scores. HW constraint: matmuls within one
    # PSUM bank must not alternate row groups -> even heads (array rows 0:64)
    # and odd heads (rows 64:128) accumulate in separate score banks, emitted
    # interleaved so PE overlaps LDWEIGHTS across row groups.
    def _softmax_T(sc, grp):
        n = len(grp)
        W = n * S
        maskT_bc = P["maskT"][:77, :].unsqueeze(1).broadcast_to([77, n, S])
        t3 = P["sm"].tile([77, 462], F32, tag="tsb", name="tsb")
        nc.vector.tensor_add(
            t3[:, :W].rearrange("p (n k) -> p n k", k=S),
            sc[:, :W].rearrange("p (n k) -> p n k", k=S),
            maskT_bc,
        )
        eb = P["sm"].tile([77, 462], BF16, tag="esb", name="esb")
        nc.scalar.activation(eb[:, :W], t3[:, :W], AF.Exp)
        # z[1, W] = column sums of eb (reduce along partitions via PE);
        # shares the score pool's PSUM banks (short-lived)
        zps = P["zz"].tile([1, 462], F32, tag="zz", name="zps")
        nc.tensor.matmul(
            zps[:, :W], P["ones"][:77, 0:1], eb[:, :W], start=True, stop=True,
        )
        zr = P["zr"].tile([1, 462], BF16, tag="zr", name="zr")
        with nc.allow_low_precision(reason="1/z in bf16: feeds bf16 att weights"):
            nc.vector.reciprocal(zr[:, :W], zps[:, :W])
        # replicate 1/z down the k-partitions with a K=1 outer product on PE
        # (GpSimd elementwise measured far slower on HW than its sim cost)
        zrep = P["zz"].tile([77, 462], F32, tag="zz", name="zrep")
        nc.tensor.matmul(
            zrep[:, :W], P["ones"][0:1, :77], zr[:, :W], start=True, stop=True,
        )
        aT = P["attT"].tile([77, 462], BF16, tag="attT", name="attT")
        nc.vector.tensor_mul(aT[:, :W], eb[:, :W], zrep[:77, :W])
        return aT

    evens = [(g, h) for g in range(G) for h in range(0, H, 2)]
    odds = [(g, h) for g in range(G) for h in range(1, H, 2)]
    egroups = [evens[i:i + 6] for i in range(0, len(evens), 6)]
    ogroups = [odds[i:i + 6] for i in range(0, len(odds), 6)]
    attT = []
    pair_loc = {}
    for eg, og in zip(egroups, ogroups):
        scA_f = P["scps"].tile([128, 512], F32, tag="sc", name="sc")
        scB_f = P["scps"].tile([128, 512], F32, tag="sc", name="sc")
        scA, scB = scA_f[:77, :], scB_f[:77, :]
        for i in range(len(eg)):
            gA, hA = eg[i]
            gB, hB = og[i]
            cA, cB = hA // 2, hB // 2
            # S^T[k, q] = K @ Q^T : lhsT = K^T chunk, rhs = Q^T chunk
            nc.tensor.matmul(
                scA[:, S * i:S * (i + 1)],
                qk[8 + cA][0:64, S * gA:S * (gA + 1)],
                qk[cA][0:64, S * gA:S * (gA + 1)],
                start=True, stop=True,
            )
            nc.tensor.matmul(
                scB[:, S * i:S * (i + 1)],
                qk[8 + cB][64:128, S * gB:S * (gB + 1)],
                qk[cB][64:128, S * gB:S * (gB + 1)],
                start=True, stop=True,
            )
        aT_A = _softmax_T(scA, eg)
        for i, pr in enumerate(eg):
            pair_loc[pr] = (len(attT), i)
        attT.append(aT_A)
        aT_B = _softmax_T(scB, og)
        for i, pr in enumerate(og):
            pair_loc[pr] = (len(attT), i)
        attT.append(aT_B)

    # ---- O^T: head-pair col-packed matmuls, PSUM bank per pair-index j
    ot = []
    for j in range(8):
        ps2 = P["m2ps"].tile([128, 512], F32, tag="m2", name="m2")
        for g in range(G):
            giE, slE = pair_loc[(g, 2 * j)]
            giO, slO = pair_loc[(g, 2 * j + 1)]
            nc.tensor.matmul(
                ps2[0:64, S * g:S * (g + 1)],
                vb[g][:, 64 * (2 * j):64 * (2 * j) + 64],
                attT[giE][:, S * slE:S * (slE + 1)],
                start=True, stop=True,
            )
            nc.tensor.matmul(
                ps2[64:128, S * g:S * (g + 1)],
                vb[g][:, 64 * (2 * j + 1):64 * (2 * j + 1) + 64],
                attT[giO][:, S * slO:S * (slO + 1)],
                start=True, stop=True,
                tile_position=(0, 64),
            )
        o = P["ot"].tile([128, Tb], BF16, tag=f"ot{j}")
        nc.scalar.activation(
            o[:], ps2[:, :Tb], AF.Identity, bias=P["bv"][:, j:j + 1]
        )
        ot.append(o)

    # ---- projection: y^T[e-chunk, t] = sum_j Wp[j-chunk]^T @ O^T[j]
    # (uses the m2 pool, not gps: keeps next block's QK GEMM from queuing
    # behind proj in the gps slot-request FIFO)
    for ec in range(8):
        ps = P["m2ps"].tile([128, 512], F32, tag="m2")
        for j in range(8):
            nc.tensor.matmul(
                ps[:, :Tb],
                P["wp"][j][:, 128 * ec:128 * (ec + 1)],
                ot[j][:],
                start=(j == 0), stop=(j == 7),
            )
        y = P["y"].tile([128, Tb], F32, tag="y")
        nc.scalar.activation(
            y[:], ps[:, :Tb], AF.Identity, bias=P["bp"][:, ec:ec + 1]
        )
        nc.sync.dma_start(P["yT"][128 * ec:128 * (ec + 1), t0:t0 + Tb], y[:])


def build(blocks=None, repeat=1):
    if blocks is None:
        blocks = BLOCKS
    nc = bacc.Bacc(None)
    xT = nc.dram_tensor("xT", [E, T], BF16, kind="ExternalInput")
    wqk_d = nc.dram_tensor("wqk", [E, 2048], BF16, kind="ExternalInput")
    wv_d = nc.dram_tensor("wv", [E, 1024], BF16, kind="ExternalInput")
    wp_d = nc.dram_tensor("wp", [1024, 1024], BF16, kind="ExternalInput")
    bqk_d = nc.dram_tensor("bqk", [128, 16], F32, kind="ExternalInput")
    bv_d = nc.dram_tensor("bv", [128, 8], F32, kind="ExternalInput")
    bp_d = nc.dram_tensor("bp", [128, 8], F32, kind="ExternalInput")
    maskT_d = nc.dram_tensor("maskT", [77, 77], F32, kind="ExternalInput")
    ones_d = nc.dram_tensor("ones", [77, 77], BF16, kind="ExternalInput")
    yT = nc.dram_tensor("yT", [E, T], F32, kind="ExternalOutput")

    with tile.TileContext(nc) as tc:
        with (
            tc.tile_pool(name="w", bufs=1) as wpool,
            tc.tile_pool(name="x", bufs=2) as xpool,
            tc.tile_pool(name="qk", bufs=2) as qkpool,
            tc.tile_pool(name="v", bufs=2) as vpool,
            tc.tile_pool(name="sm", bufs=3) as smpool,
            tc.tile_pool(name="zr", bufs=3) as zrpool,
            tc.tile_pool(name="attT", bufs=18) as attTpool,
            tc.tile_pool(name="ot", bufs=2) as otpool,
            tc.tile_pool(name="y", bufs=3) as ypool,
            tc.tile_pool(name="gps", bufs=2, space="PSUM") as gpspool,
            tc.tile_pool(name="scps", bufs=2, space="PSUM") as scpool,
            tc.tile_pool(name="zz", bufs=2, space="PSUM") as zzpool,
            tc.tile_pool(name="m2ps", bufs=2, space="PSUM") as m2pool,
        ):
            P = {}
            # DMA order matters: small constants + first QK weight chunk
            # first so block 0's x tiles (emitted next, in _emit_block) don't
            # queue behind 8MB of weights; remaining weights stream in while
            # block 0's QK GEMM runs.
            P["bqk"] = wpool.tile([128, 16], F32, tag="bqk", name="bqk")
            nc.sync.dma_start(P["bqk"][:], bqk_d[:])
            P["bv"] = wpool.tile([128, 8], F32, tag="bv", name="bv")
            nc.sync.dma_start(P["bv"][:], bv_d[:])
            P["bp"] = wpool.tile([128, 8], F32, tag="bp", name="bp")
            nc.sync.dma_start(P["bp"][:], bp_d[:])
            P["maskT"] = wpool.tile([77, 77], F32, tag="maskT", name="maskT")
            nc.sync.dma_start(P["maskT"][:], maskT_d[:])
            P["ones"] = wpool.tile([77, 77], BF16, tag="ones", name="ones")
            nc.sync.dma_start(P["ones"][:], ones_d[:])
            P["xT"] = xT
            P["x"] = xpool
            xt0 = _load_x(nc, P, blocks[0][0], blocks[0][1]) if repeat == 1 else None
            P["wqk"] = []
            P["wv"] = []
            P["wp"] = []
            for e in range(8):
                w1 = wpool.tile([128, 2048], BF16, tag=f"wqk{e}", name=f"wqk{e}")
                nc.sync.dma_start(w1[:], wqk_d[128 * e:128 * (e + 1), :])
                P["wqk"].append(w1)
            for e in range(8):
                w2 = wpool.tile([128, 1024], BF16, tag=f"wv{e}", name=f"wv{e}")
                nc.sync.dma_start(w2[:], wv_d[128 * e:128 * (e + 1), :])
                P["wv"].append(w2)
            for e in range(8):
                w3 = wpool.tile([128, 1024], BF16, tag=f"wp{e}", name=f"wp{e}")
                nc.sync.dma_start(w3[:], wp_d[128 * e:128 * (e + 1), :])
                P["wp"].append(w3)
            P["yT"] = yT
            P["qk"] = qkpool
            P["v"] = vpool
            P["sm"] = smpool
            P["zr"] = zrpool
            P["zz"] = zzpool
            P["attT"] = attTpool
            P["ot"] = otpool
            P["y"] = ypool
            P["gps"] = gpspool
            P["scps"] = scpool
            P["m2ps"] = m2pool

            def body(first_xt=None):
                for bi, (b0, G) in enumerate(blocks):
                    _emit_block(nc, tc, P, b0, G,
                                xt=first_xt if bi == 0 else None)

            if repeat == 1:
                body(first_xt=xt0)
            else:
                # first iteration reuses the prefetched x tiles; the loop
                # reloads them each pass (identical work every iteration)
                with tc.For_i(0, repeat):
                    body()

    nc.finalize()
    return nc


_CACHE = {}


def _get_nc():
    if "nc" not in _CACHE:
        _CACHE["nc"] = build()
    return _CACHE["nc"]


def make_inputs(x, W_attn, b_attn, W_proj, b_proj):
    """Host-side prep: shard + transpose + cast. Returns per-core input maps."""
    x = np.asarray(x, dtype=np.float32)
    W_attn = np.asarray(W_attn, dtype=np.float32)
    b_attn = np.asarray(b_attn, dtype=np.float32)
    W_proj = np.asarray(W_proj, dtype=np.float32)
    b_proj = np.asarray(b_proj, dtype=np.float32)

    wqk = W_attn[:, :2048].astype(ml_dtypes.bfloat16)
    wv = W_attn[:, 2048:].astype(ml_dtypes.bfloat16)
    wp = W_proj.astype(ml_dtypes.bfloat16)
    # bias chunks [128, 16]: col c = b_attn[128c:128c+128]; Q part pre-scaled
    bq = b_attn[:2048].copy()
    bq[:1024] *= SCALE
    bqk = np.stack([bq[128 * c:128 * (c + 1)] for c in range(16)], axis=1).astype(np.float32)
    bv = np.stack([b_attn[2048 + 128 * j:2048 + 128 * (j + 1)] for j in range(8)], axis=1).astype(np.float32)
    bp = np.stack([b_proj[128 * c:128 * (c + 1)] for c in range(8)], axis=1).astype(np.float32)
    # transposed causal mask: maskT[k, q] = 0 if k <= q else NEG
    maskT = np.where(
        np.triu(np.ones((77, 77), dtype=bool)), 0.0, NEG
    ).astype(np.float32)
    ones = np.ones((77, 77), dtype=ml_dtypes.bfloat16)

    maps = []
    for cid in range(N_CORES):
        xs = x[BC * cid:BC * (cid + 1)].reshape(T, E)
        xTc = np.ascontiguousarray(xs.T).astype(ml_dtypes.bfloat16)
        maps.append({
            "xT": xTc, "wqk": wqk, "wv": wv, "wp": wp,
            "bqk": bqk, "bv": bv, "bp": bp, "maskT": maskT, "ones": ones,
        })
    return maps


def assemble_output(results):
    y = np.empty((B, S, E), dtype=np.float32)
    for cid in range(N_CORES):
        yTc = results[cid]["yT"]  # [E, T]
        y[BC * cid:BC * (cid + 1)] = yTc.T.reshape(BC, S, E)
    return y


def kernel(x, W_attn, b_attn, W_proj, b_proj):
    nc = _get_nc()
    maps = make_inputs(x, W_attn, b_attn, W_proj, b_proj)
    res = run_bass_kernel_spmd(nc, maps, list(range(N_CORES)))
    return assemble_output(res.results)
